# revision 47
# baseline (speedup 1.0000x reference)
"""Trainium2 Bass kernel for nn_BetweenClusterFC.

Computes out[e] = (emb_1[f[e]] @ W1 + b1) . (emb_2[t[e]] @ W2 + b2)
for E = 1.6M edges over N = 100k nodes, D_IN = 256, D_OUT = 128.

Strategy (8 NeuronCores, SPMD, full inputs in / full output out):
  - Nodes are split into 8 blocks of 12500 (padded to 12800).  Edges are
    assigned to cores by a (from-block-group, to-block-group) 2x4 rectangle:
    core c=(a,b) handles edges with from-node in blocks [4a..4a+3] and
    to-node in blocks [2b..2b+1].  Each core projects its 6 blocks in bf16
    on the PE (bias folded into the accumulation as a rank-1 matmul),
    drains PSUM to bf16 (DVE early / Activation engine later), and writes
    bf16 projection tables to local DRAM in a partition-major permuted row
    layout so each table write is a single fully-contiguous DMA.
  - Edges are bucketed host-side by (local from-block, local to-block);
    per bucket both endpoint rows are fetched with SWDGE dma_gather calls
    (int16 permuted indices, 256B bf16 rows) sized exactly to the bucket,
    then a DVE bf16 multiply + add-tree + reduce produces the per-edge
    dot products.
  - The host applies the inverse edge permutation to assemble the output.

Per-queue budget: Pool (gathers) ~335us is the critical queue; DVE ~300us;
SP/Act share HWDGE DMA traffic; PE ~100us.

Written in raw Bass (explicit semaphores) — the Tile layer's generated
sync exceeds this toolchain's per-instruction wait-slot limits.
"""

import contextlib

import numpy as np
import ml_dtypes

import concourse.bass as bass
import concourse.mybir as mybir

# ---------------------------------------------------------------- constants
N_NODES = 100_000
D_IN = 256
D_OUT = 128
N_EDGES = 1_600_000
N_CORES = 8

NB = 12_500          # nodes per block
NBP = 12_800         # padded block rows (25 * 512)
NFB = 4              # from-blocks per core
NTB = 2              # to-blocks per core
NBUCKET = NFB * NTB  # 8 buckets per core

P1_ROWS = NFB * NBP  # 51200
P2_ROWS = NTB * NBP  # 25600

TILES1 = P1_ROWS // 128    # 400 node-tiles, table 1
TILES2 = P2_ROWS // 128    # 200 node-tiles, table 2
GROUPS1 = TILES1 // 4      # 100 psum groups (512 rows each)
GROUPS2 = TILES2 // 4      # 50
NGROUP = GROUPS1 + GROUPS2  # 150
CHUNK_T = 20               # node-tiles per embT load chunk
NCH1 = TILES1 // CHUNK_T   # 20 chunks
NCH2 = TILES2 // CHUNK_T   # 10
NCHUNK = NCH1 + NCH2       # 30
EMB_COLS = CHUNK_T * 128   # 2560
GPC = CHUNK_T // 4         # 5 groups per chunk

F32 = mybir.dt.float32
BF16 = mybir.dt.bfloat16
I16 = mybir.dt.int16
AX = mybir.AxisListType
BFNP = ml_dtypes.bfloat16

# Group processing order: alternate p2/p1 blocks so the first gather bucket
# (from-block 0, to-block 0) becomes ready after only 50 groups.
# p1 block fi = groups [25*fi, 25*fi+25); p2 block ti = groups [100+25*ti, ...).
GSEQ = (list(range(100, 125)) + list(range(0, 25)) +
        list(range(125, 150)) + list(range(25, 50)) +
        list(range(50, 75)) + list(range(75, 100)))
# chunk ids: 0..19 table1 (5 per block), 20..29 table2
CSEQ = (list(range(20, 25)) + list(range(0, 5)) +
        list(range(25, 30)) + list(range(5, 10)) +
        list(range(10, 15)) + list(range(15, 20)))
CPOS = {cid: q for q, cid in enumerate(CSEQ)}
# processed-group count after which bucket bk=(fi*NTB+ti) may gather
READY_Q = [50, 75, 100, 100, 125, 125, 150, 150]
DRAIN_SPLIT = 50  # groups < split drain on DVE (idle early), rest on Act


def _chunk_of_tile(tg):
    """global tile index -> (global chunk id, local col0)."""
    if tg < TILES1:
        return tg // CHUNK_T, (tg % CHUNK_T) * 128
    t2 = tg - TILES1
    return NCH1 + t2 // CHUNK_T, (t2 % CHUNK_T) * 128


def _chunk_src(c):
    """global chunk id -> (table, col0)."""
    if c < NCH1:
        return 0, c * EMB_COLS
    return 1, (c - NCH1) * EMB_COLS


def _make_calls(cap):
    calls = [1024] * (cap // 1024)
    if cap % 1024:
        calls.append(cap % 1024)
    return calls


def _layout(caps):
    """Per-bucket call/slot/col bookkeeping shared by device + host code."""
    bcalls = [_make_calls(c) for c in caps]
    bslots = [[g // 128 for g in cl] for cl in bcalls]
    bcols = [[g // 16 for g in cl] for cl in bcalls]
    bslot_tot = [c // 128 for c in caps]
    bcol_tot = [c // 16 for c in caps]
    col_off = np.concatenate([[0], np.cumsum(bcol_tot)]).astype(int)
    slot_max = max(bslot_tot)
    flat = [(bk, ci) for bk in range(NBUCKET) for ci in range(len(bcalls[bk]))]
    return bcalls, bslots, bcols, bslot_tot, col_off, slot_max, flat


# ---------------------------------------------------------------- device code
def build_bass(caps):
    bcalls, bslots, bcols, bslot_tot, col_off, slot_max, flat = _layout(caps)
    idx_cols = int(col_off[-1])
    ncall = len(flat)
    # reduce count after which bucket bk's rt tile is complete
    red_done = np.cumsum([len(c) for c in bcalls]).astype(int)

    nc = bass.Bass()

    e1t = nc.dram_tensor("e1t", [D_IN, P1_ROWS], BF16, kind="ExternalInput")
    e2t = nc.dram_tensor("e2t", [D_IN, P2_ROWS], BF16, kind="ExternalInput")
    w1 = nc.dram_tensor("w1", [D_IN, D_OUT], BF16, kind="ExternalInput")
    w2 = nc.dram_tensor("w2", [D_IN, D_OUT], BF16, kind="ExternalInput")
    b1f = nc.dram_tensor("b1f", [1, 512], BF16, kind="ExternalInput")
    b2f = nc.dram_tensor("b2f", [1, 512], BF16, kind="ExternalInput")
    b1t = nc.dram_tensor("b1t", [128, 512], BF16, kind="ExternalInput")
    b2t = nc.dram_tensor("b2t", [128, 512], BF16, kind="ExternalInput")
    onesd = nc.dram_tensor("onesd", [1, 128], BF16, kind="ExternalInput")
    idxa = nc.dram_tensor("idxa", [128, idx_cols], I16, kind="ExternalInput")
    idxb = nc.dram_tensor("idxb", [128, idx_cols], I16, kind="ExternalInput")
    res = nc.dram_tensor("res", [NBUCKET, 128, slot_max], BF16,
                         kind="ExternalOutput")

    p1d = nc.dram_tensor("p1d", [P1_ROWS, D_OUT], BF16, kind="Internal")
    p2d = nc.dram_tensor("p2d", [P2_ROWS, D_OUT], BF16, kind="Internal")
    pdst = (p1d, p2d)

    st = contextlib.ExitStack()
    with st:
        sb = lambda nm, shape, dt=BF16: st.enter_context(nc.sbuf_tensor(nm, shape, dt))
        sem = lambda nm: st.enter_context(nc.semaphore(name=nm))

        w1c = sb("w1c", [128, 256])
        w2c = sb("w2c", [128, 256])
        bt = (sb("bt1", [1, 512]), sb("bt2", [1, 512]))
        btile = (sb("btile1", [128, 512]), sb("btile2", [128, 512]))
        onesr = sb("onesr", [1, 128])
        idxt = (sb("idxta", [128, idx_cols], I16), sb("idxtb", [128, idx_cols], I16))
        et = [[sb(f"et_{p}_{k}", [128, EMB_COLS]) for k in range(2)]
              for p in range(4)]  # [chunk mod 4][din-half]
        pvall = sb("pvall", [128, 16 * 512])
        pv = [pvall[:, i * 512:(i + 1) * 512] for i in range(16)]
        ps = [st.enter_context(nc.psum_tensor(f"ps{i}", [128, 512], F32))
              for i in range(8)]
        at = [sb(f"at{i}", [128, 8 * 128]) for i in range(8)]
        btg = [sb(f"btg{i}", [128, 8 * 128]) for i in range(8)]
        rt = [sb(f"rt{i}", [128, slot_max]) for i in range(4)]

        s_cl = sem("s_cl")               # bt/onesr consts (3 dmas -> 48)
        s_cw = sem("s_cw")               # W tiles (4 dmas -> 64)
        s_cb = sem("s_cb")               # bias tiles (2 dmas -> 32)
        s_idx = tuple(sem(f"s_idx{b}") for b in range(NBUCKET))  # 2 dmas -> 32
        s_load = tuple(sem(f"s_load{i}") for i in range(4))  # embT, by chunk%4
        s_mm = sem("s_mm")               # tile matmuls (+1 each; 2 per tile)
        s_bias = sem("s_bias")           # bias matmuls (+1 per region; 4/group)
        s_dd = sem("s_dd")               # DVE drains (groups < DRAIN_SPLIT)
        s_da = sem("s_da")               # Act drains (groups >= DRAIN_SPLIT)
        s_pw = tuple(sem(f"s_pw{i}") for i in range(8))   # SP p-writes (q>=50), by g%8
        s_pwp = tuple(sem(f"s_pwp{i}") for i in range(8))  # Pool p-writes (q<50)
        s_g = tuple(sem(f"s_g{i}") for i in range(8))  # gathers, by k%8
        s_mul = sem("s_mul")             # muls (+1 per call)
        s_red = sem("s_red")             # final reduces (+1 per call)
        s_out = tuple(sem(f"s_out{i}") for i in range(4))  # res dmas, by bk%4

        CONSTS = 9 * 16

        block = st.enter_context(nc.Block())

        # early p-write pairing plan: two adjacent 512-row groups per DMA
        # where pv slots and DRAM rows are both adjacent
        pw_dmas = []
        pw_jmap = {}
        qq = 0
        while qq < DRAIN_SPLIT:
            pair = (qq + 1 < DRAIN_SPLIT and (qq % 16) + 1 < 16
                    and (qq % 25) + 1 < 25)
            pw_dmas.append((qq, 2 if pair else 1))
            pw_jmap[qq] = len(pw_dmas) - 1
            if pair:
                pw_jmap[qq + 1] = len(pw_dmas) - 1
            qq += 2 if pair else 1
        pw_count = {p: (pw_jmap[p] % 8,
                        sum(1 for j2 in range(pw_jmap[p] + 1)
                            if j2 % 8 == pw_jmap[p] % 8))
                    for p in pw_jmap}


        def wait_write(eng, p):
            if p < DRAIN_SPLIT:
                r, n = pw_count[p]
                eng.wait_ge(s_pwp[r], 16 * n)
            else:
                r = p % 8
                n = len([x for x in range(DRAIN_SPLIT, p + 1) if x % 8 == r])
                eng.wait_ge(s_pw[r], 16 * n)

        def wait_drained(eng, g):
            """Wait until group g's PSUM->pv drain has completed."""
            if g < DRAIN_SPLIT:
                eng.wait_ge(s_dd, g + 1)
            else:
                eng.wait_ge(s_da, g - DRAIN_SPLIT + 1)

        def make_load_chunk(eng, half):
            def load_chunk(cq):
                if cq >= 4:
                    # buffer cq%4 held chunk cq-4; wait until PE consumed it
                    eng.wait_ge(s_mm, 2 * CHUNK_T * (cq - 3))
                tab, col0 = _chunk_src(CSEQ[cq])
                src = e1t if tab == 0 else e2t
                eng.dma_start(
                    out=et[cq % 4][half][:],
                    in_=src[half * 128:(half + 1) * 128, col0:col0 + EMB_COLS],
                ).then_inc(s_load[cq % 4], 16)
            return load_chunk

        def run_load_loop(eng, half, body):
            """Shared SP/Act structure: per-group work + lookahead chunk loads.

            body(q) comes FIRST so the chunk-reuse wait (on PE progress, which
            transitively needs this engine's body work) cannot deadlock."""
            load_chunk = make_load_chunk(eng, half)
            load_chunk(0)
            load_chunk(1)
            next_cq = 2
            for q in range(NGROUP):
                body(q)
                while next_cq < NCHUNK and next_cq * GPC <= q + 2 * GPC:
                    load_chunk(next_cq)
                    next_cq += 1

        # ------------------------------------------------ SP: consts, embT
        # half-0, p-table writes, res stores
        @block.sync
        def _(sync):
            for k in range(2):
                sync.dma_start(out=w1c[:, k * 128:(k + 1) * 128],
                               in_=w1[k * 128:(k + 1) * 128, :]).then_inc(s_cw, 16)
                sync.dma_start(out=w2c[:, k * 128:(k + 1) * 128],
                               in_=w2[k * 128:(k + 1) * 128, :]).then_inc(s_cw, 16)
            sync.dma_start(out=btile[0][:], in_=b1t[:]).then_inc(s_cb, 16)
            sync.dma_start(out=btile[1][:], in_=b2t[:]).then_inc(s_cb, 16)
            sync.dma_start(out=bt[0][:], in_=b1f[:]).then_inc(s_cl, 16)
            sync.dma_start(out=bt[1][:], in_=b2f[:]).then_inc(s_cl, 16)
            sync.dma_start(out=onesr[:], in_=onesd[:]).then_inc(s_cl, 16)

            sp_seen = [0] * 8

            def p_write(eng, q):
                g = GSEQ[q]
                wait_drained(eng, q)
                if sp_seen[q % 8]:
                    eng.wait_ge(s_pw[q % 8], 16 * sp_seen[q % 8])
                sp_seen[q % 8] += 1
                tab = 0 if g < GROUPS1 else 1
                gl = g if tab == 0 else g - GROUPS1
                # permuted row layout: group gl's 512 rows are stored
                # partition-major (4 consecutive rows per partition), so the
                # write is one fully-contiguous [128 x 1024B] slab.
                eng.dma_start(
                    out=pdst[tab][gl * 512:(gl + 1) * 512, :]
                        .rearrange("(p j) d -> p (j d)", p=128),
                    in_=pv[q % 16],
                ).then_inc(s_pw[q % 8], 16)

            def body(q):
                # early p-writes run on the (otherwise idle) Pool queue to
                # unclog the startup chain; SP takes over after DRAIN_SPLIT
                if q >= DRAIN_SPLIT:
                    p_write(sync, q)

            run_load_loop(sync, 0, body)

            for bk in range(NBUCKET):
                sync.wait_ge(s_red, int(red_done[bk]))
                stot = bslot_tot[bk]
                sync.dma_start(out=res[bk][:, 0:stot],
                               in_=rt[bk % 4][:, 0:stot]).then_inc(s_out[bk % 4], 16)
            for r in range(4):
                sync.wait_ge(s_out[r], 16 * len(range(r, NBUCKET, 4)))

        # ------------------------------------------------ Act: embT half-1,
        # JIT idx pieces, late PSUM -> bf16 drains
        @block.scalar
        def _(scalar):
            def body(q):
                bk = None
                if q in (1, 4):
                    bk = (q - 1) // 3
                elif q >= DRAIN_SPLIT + 2 and (q - DRAIN_SPLIT - 2) % 3 == 0:
                    b2 = 2 + (q - DRAIN_SPLIT - 2) // 3
                    bk = b2 if b2 < NBUCKET else None
                if bk is not None:
                    c0, c1 = int(col_off[bk]), int(col_off[bk + 1])
                    scalar.dma_start(out=idxt[0][:, c0:c1],
                                     in_=idxa[:, c0:c1]).then_inc(s_idx[bk], 16)
                    scalar.dma_start(out=idxt[1][:, c0:c1],
                                     in_=idxb[:, c0:c1]).then_inc(s_idx[bk], 16)
                if q < DRAIN_SPLIT:
                    return
                scalar.wait_ge(s_bias, 4 * (q - DRAIN_SPLIT + 1))
                if q >= 16:
                    wait_write(scalar, q - 16)  # pv[q%16] free
                scalar.activation(
                    out=pv[q % 16], in_=ps[q % 8][:],
                    func=mybir.ActivationFunctionType.Copy,
                ).then_inc(s_da, 1)

            run_load_loop(scalar, 1, body)

        # ------------------------------------------------ PE: projections
        @block.tensor
        def _(tensor):
            tensor.wait_ge(s_cw, 4 * 16)  # W tiles
            for q, g in enumerate(GSEQ):
                if q == DRAIN_SPLIT:
                    tensor.wait_ge(s_cl, 3 * 16)  # bt/onesr for bias matmuls
                tab = 0 if g < GROUPS1 else 1
                wc = w1c if tab == 0 else w2c
                if q >= 8:
                    wait_drained(tensor, q - 8)  # psum bank q%8 free
                for j in range(4):
                    tg = g * 4 + j if tab == 0 else TILES1 + (g - GROUPS1) * 4 + j
                    cid, col0 = _chunk_of_tile(tg)
                    cq = CPOS[cid]
                    if tg % CHUNK_T == 0:  # first tile of chunk
                        tensor.wait_ge(s_load[cq % 4], 32 * (cq // 4 + 1))
                    out = ps[q % 8][:, j * 128:(j + 1) * 128]
                    late_bias = q >= DRAIN_SPLIT
                    tensor.matmul(out=out, lhsT=et[cq % 4][0][:, col0:col0 + 128],
                                  rhs=wc[:, 0:128], start=True, stop=False).then_inc(s_mm, 1)
                    tensor.matmul(out=out, lhsT=et[cq % 4][1][:, col0:col0 + 128],
                                  rhs=wc[:, 128:256], start=False,
                                  stop=not late_bias).then_inc(s_mm, 1)
                    if late_bias:
                        # bias as a rank-1 accumulation closing the group
                        tensor.matmul(out=out, lhsT=onesr[:],
                                      rhs=bt[tab][:, j * 128:(j + 1) * 128],
                                      start=False, stop=True).then_inc(s_bias, 1)

        # ------------------------------------------------ DVE: early drains,
        # then dot products (bf16 mul + add-tree + short reduce)
        @block.vector
        def _(vector):
            vector.wait_ge(s_cb, 2 * 16)  # bias tiles
            for q in range(DRAIN_SPLIT):
                vector.wait_ge(s_mm, 8 * (q + 1))
                if q >= 16:
                    wait_write(vector, q - 16)
                tab = 0 if GSEQ[q] < GROUPS1 else 1
                vector.tensor_add(out=pv[q % 16], in0=ps[q % 8][:],
                                  in1=btile[tab][:]).then_inc(s_dd, 1)

            with nc.allow_low_precision(reason="bf16 dot-product tree reduce; "
                                        "tolerance 2e-2"):
                for k, (bk, ci) in enumerate(flat):
                    S = bslots[bk][ci]
                    scol = sum(bslots[bk][:ci])
                    vector.wait_ge(s_g[k % 8], 32 * (k // 8 + 1))
                    if ci == 0 and bk >= 4:
                        vector.wait_ge(s_out[bk % 4], 16 * (bk // 4))  # rt drained
                    a3 = at[k % 8][:, :S * 128]
                    b3 = btg[k % 8][:, :S * 128]
                    vector.tensor_mul(out=a3, in0=a3, in1=b3).then_inc(s_mul, 1)
                    vector.wait_ge(s_mul, 3 * k + 1)
                    w2v = a3.rearrange("p (s t d) -> p s t d", t=2, d=64)
                    vector.tensor_add(out=w2v[:, :, 0:1, :], in0=w2v[:, :, 0:1, :],
                                      in1=w2v[:, :, 1:2, :]).then_inc(s_mul, 1)
                    vector.wait_ge(s_mul, 3 * k + 2)
                    w4v = a3.rearrange("p (s t d) -> p s t d", t=4, d=32)
                    vector.tensor_add(out=w4v[:, :, 0:1, :], in0=w4v[:, :, 0:1, :],
                                      in1=w4v[:, :, 1:2, :]).then_inc(s_mul, 1)
                    vector.wait_ge(s_mul, 3 * k + 3)
                    vector.reduce_sum(
                        out=rt[bk % 4][:, scol:scol + S].rearrange("p (s o) -> p s o", o=1),
                        in_=w4v[:, :, 0:1, :],
                        axis=AX.X,
                    ).then_inc(s_red, 1)

        # ------------------------------------------------ Pool: gathers
        @block.gpsimd
        def _(gpsimd):
            from concourse import library_config
            gpsimd.load_library(library_config.mlp)
            sizes = sorted({gsz for cl in bcalls for gsz in cl})
            regs = {gsz: gpsimd.to_reg(gsz) for gsz in sizes}
            pw_seen = [0] * 8

            def emit_pw(j, q0, n):
                g = GSEQ[q0]
                gpsimd.wait_ge(s_dd, q0 + n)
                if pw_seen[j % 8]:
                    # serialize same-sem updates (keeps waiters race-free)
                    gpsimd.wait_ge(s_pwp[j % 8], 16 * pw_seen[j % 8])
                pw_seen[j % 8] += 1
                tab = 0 if g < GROUPS1 else 1
                gl = g if tab == 0 else g - GROUPS1
                if n == 2:
                    gpsimd.dma_start(
                        out=pdst[tab][gl * 512:(gl + 2) * 512, :]
                            .rearrange("(g p j) d -> p g (j d)", g=2, p=128),
                        in_=pvall[:, (q0 % 16) * 512:((q0 % 16) + 2) * 512]
                            .rearrange("p (g x) -> p g x", g=2),
                    ).then_inc(s_pwp[j % 8], 16)
                else:
                    gpsimd.dma_start(
                        out=pdst[tab][gl * 512:(gl + 1) * 512, :]
                            .rearrange("(p j) d -> p (j d)", p=128),
                        in_=pv[q0 % 16],
                    ).then_inc(s_pwp[j % 8], 16)

            for j, (q0, n) in enumerate(pw_dmas):
                emit_pw(j, q0, n)
            gated = -1
            for k, (bk, ci) in enumerate(flat):
                if ci == 0:
                    gpsimd.wait_ge(s_idx[bk], 32)
                    if READY_Q[bk] > gated:
                        gated = READY_Q[bk]
                        need = [0] * 8
                        for p in range(min(gated, DRAIN_SPLIT)):
                            r, n = pw_count[p]
                            need[r] = max(need[r], n)
                        for r in range(8):
                            if need[r]:
                                gpsimd.wait_ge(s_pwp[r], 16 * need[r])
                            ns_ = len([x for x in range(DRAIN_SPLIT, gated)
                                       if x % 8 == r])
                            if ns_:
                                gpsimd.wait_ge(s_pw[r], 16 * ns_)
                fi, ti = bk // NTB, bk % NTB
                gsz = bcalls[bk][ci]
                S = bslots[bk][ci]
                col0 = int(col_off[bk]) + sum(bcols[bk][:ci])
                ncols = bcols[bk][ci]
                if k >= 8:
                    gpsimd.wait_ge(s_red, k - 7)  # at/btg[k%8] consumed
                gpsimd.dma_gather(
                    out_ap=at[k % 8][:, :S * 128].rearrange("p (s d) -> p s d", d=128),
                    in_ap=p1d[fi * NBP:(fi + 1) * NBP, :],
                    idxs_ap=idxt[0][:, col0:col0 + ncols],
                    num_idxs=gsz, num_idxs_reg=regs[gsz], elem_size=D_OUT,
                    queue_num=0,
                ).then_inc(s_g[k % 8], 16)
                gpsimd.dma_gather(
                    out_ap=btg[k % 8][:, :S * 128].rearrange("p (s d) -> p s d", d=128),
                    in_ap=p2d[ti * NBP:(ti + 1) * NBP, :],
                    idxs_ap=idxt[1][:, col0:col0 + ncols],
                    num_idxs=gsz, num_idxs_reg=regs[gsz], elem_size=D_OUT,
                    queue_num=0,
                ).then_inc(s_g[k % 8], 16)

    return nc


_NC_CACHE = {}


def _get_nc(caps=None):
    global _NC_CACHE
    if caps is None:
        assert _NC_CACHE, "call _marshal first to determine caps"
        return next(iter(_NC_CACHE.values()))
    caps = tuple(caps)
    if caps not in _NC_CACHE:
        nc = build_bass(caps)
        from concourse.library_overlay import lower_extended_insts
        lower_extended_insts(nc)
        _NC_CACHE[caps] = nc
    return _NC_CACHE[caps]


# ---------------------------------------------------------------- host side
def _perm_rows(n):
    """local node id -> permuted table row (partition-major within 512)."""
    return (n // 512) * 512 + (n % 128) * 4 + (n % 512) // 128


def _marshal(emb_1, emb_2, nodes_from_to, W1, b1, W2, b2):
    """Shard/bucket inputs per core.  Returns (in_maps, books, caps)."""
    f = np.asarray(nodes_from_to[:, 0], dtype=np.int64)
    t = np.asarray(nodes_from_to[:, 1], dtype=np.int64)
    e1T = np.ascontiguousarray(
        np.asarray(emb_1, dtype=np.float32).T).astype(BFNP)
    e2T = np.ascontiguousarray(
        np.asarray(emb_2, dtype=np.float32).T).astype(BFNP)
    W1 = np.asarray(W1, dtype=np.float32).astype(BFNP)
    W2 = np.asarray(W2, dtype=np.float32).astype(BFNP)
    b1 = np.asarray(b1, dtype=np.float32).reshape(-1)
    b2 = np.asarray(b2, dtype=np.float32).reshape(-1)

    core = (f // (NFB * NB)) * 4 + t // (NTB * NB)
    order0 = np.argsort(core, kind="stable")
    ccnt = np.bincount(core, minlength=N_CORES)
    coff = np.concatenate([[0], np.cumsum(ccnt)])

    b1f = np.tile(b1.reshape(1, D_OUT), (1, 4)).astype(BFNP)
    b2f = np.tile(b2.reshape(1, D_OUT), (1, 4)).astype(BFNP)
    b1tt = np.tile(b1.reshape(1, D_OUT), (128, 4)).astype(BFNP)
    b2tt = np.tile(b2.reshape(1, D_OUT), (128, 4)).astype(BFNP)
    onesd = np.ones((1, 128), BFNP)

    # first pass: per-bucket counts across cores fix the shared program shape
    pre = []
    bmax = np.zeros(NBUCKET, np.int64)
    for c in range(N_CORES):
        a, b = c // 4, c % 4
        sel = order0[coff[c]:coff[c + 1]]
        fc, tcv = f[sel], t[sel]
        fi = fc // NB - NFB * a
        ti = tcv // NB - NTB * b
        fl = _perm_rows(fc % NB).astype(np.int16)
        tl = _perm_rows(tcv % NB).astype(np.int16)
        bk = fi * NTB + ti
        o2 = np.argsort(bk, kind="stable")
        cnts = np.bincount(bk, minlength=NBUCKET)
        bmax = np.maximum(bmax, cnts)
        pre.append((sel[o2], fl[o2], tl[o2], cnts))
    caps = tuple(int(-(-m // 128) * 128) for m in bmax)
    bcalls, bslots, bcols, bslot_tot, col_off, slot_max, flat = _layout(caps)
    idx_cols = int(col_off[-1])

    in_maps, books = [], []
    for c in range(N_CORES):
        a, b = c // 4, c % 4
        sel2, fl2, tl2, cnts = pre[c]
        pos = np.concatenate([[0], np.cumsum(cnts)])

        idxa = np.zeros((128, idx_cols), np.int16)
        idxb = np.zeros((128, idx_cols), np.int16)
        for k in range(NBUCKET):
            cap_k = caps[k]
            sa = np.zeros(cap_k, np.int16)
            sbv = np.zeros(cap_k, np.int16)
            sa[:cnts[k]] = fl2[pos[k]:pos[k + 1]]
            sbv[:cnts[k]] = tl2[pos[k]:pos[k + 1]]
            # wrap by 16: idx i at (partition i%16, col i//16), replicated
            # across the 8 groups of 16 partitions
            wa = np.tile(sa.reshape(cap_k // 16, 16).T, (8, 1))
            wb = np.tile(sbv.reshape(cap_k // 16, 16).T, (8, 1))
            idxa[:, col_off[k]:col_off[k + 1]] = wa
            idxb[:, col_off[k]:col_off[k + 1]] = wb

        e1t = np.zeros((D_IN, P1_ROWS), BFNP)
        for i in range(NFB):
            blk = e1T[:, (NFB * a + i) * NB:(NFB * a + i + 1) * NB]
            e1t[:, i * NBP:i * NBP + NB] = blk
        e2t = np.zeros((D_IN, P2_ROWS), BFNP)
        for i in range(NTB):
            blk = e2T[:, (NTB * b + i) * NB:(NTB * b + i + 1) * NB]
            e2t[:, i * NBP:i * NBP + NB] = blk

        in_maps.append({
            "e1t": e1t, "e2t": e2t,
            "w1": W1, "w2": W2, "b1f": b1f, "b2f": b2f,
            "b1t": b1tt, "b2t": b2tt, "onesd": onesd,
            "idxa": idxa, "idxb": idxb,
        })
        books.append((sel2, cnts, pos))
    return in_maps, books, caps


def _unmarshal(results, books, caps, n_edges):
    bcalls, bslots, bcols, bslot_tot, col_off, slot_max, flat = _layout(caps)
    out = np.empty(n_edges, np.float32)
    for c in range(N_CORES):
        sel2, cnts, pos = books[c]
        r = np.asarray(results[c]["res"]).astype(np.float32)
        for k in range(NBUCKET):
            if cnts[k] == 0:
                continue
            arr = r[k]
            scol0 = np.concatenate([[0], np.cumsum(bslots[k])]).astype(int)
            stream = np.concatenate([
                arr[:, scol0[ci]:scol0[ci] + bslots[k][ci]].T.reshape(-1)
                for ci in range(len(bcalls[k]))
            ])
            out[sel2[pos[k]:pos[k + 1]]] = stream[:cnts[k]]
    return out


def _run(inputs, trace=False, **run_kwargs):
    from concourse.bass_utils import run_bass_kernel_spmd

    in_maps, books, caps = _marshal(**inputs)
    nc = _get_nc(caps)
    r = run_bass_kernel_spmd(
        nc, in_maps, core_ids=list(range(N_CORES)), trace=trace, **run_kwargs
    )
    out = _unmarshal(r.results, books, caps, len(inputs["nodes_from_to"]))
    return out, r


def kernel(**inputs) -> np.ndarray:
    out, _ = _run(inputs, trace=False)
    return out
